# revision 40
# baseline (speedup 1.0000x reference)
"""BWGNN (Bernstein-wavelet GNN) Trainium2 kernel, 8-core SPMD.

Sharding: nodes split 8 ways (graph/data parallel); edges partitioned by dst
shard; tiny weights replicated.  Per round of Laplacian message passing the
node-state table (dinv * f, bf16, rows padded to 256B) is distributed with
AllGather, then per-edge src rows are fetched with dma_gather (int16
indices, 1024 per instruction).

Gather descriptor generation on the Pool engine (~7.9ns/idx ucode) is the
hard bottleneck, so everything else hides under it:
- edge-slot streams are padded only per chunk (max over cores, ~0.7%);
  windows float: each canonical tile carries up to 4 candidate dst windows
  (union over cores), turned into one-hot indicators on the VectorEngine
  (iota is_eq vs 4 host bf16 dstw streams, self-masking), accumulated per
  window event on the TensorEngine in PSUM, first-touch-copied into an
  SBUF fp32 agg (no memset).
- the table is split in two parts (dst windows 0-47 / 48-97+zero row) with
  separate AllGathers, so part 0's collective overlaps the previous
  round's compute; bundles are interleaved across the 4 (part, core-half)
  chunks so windows complete progressively, and per 4-window group the
  next round's table mult + DMA (and in the last round the Bernstein
  filters + output MLP + output DMA) are emitted in-round.
All matmul operands are bf16 (PSUM fp32; agg, dinv, biases, outputs fp32).
"""

import sys
from contextlib import ExitStack

import numpy as np
import ml_dtypes

try:
    import concourse  # noqa: F401
except ImportError:  # pragma: no cover
    sys.path.insert(0, "/opt/trn_rl_repo")

import concourse.bacc as bacc
import concourse.bass as bass
import concourse.mybir as mybir
import concourse.tile as tile
from concourse.bass_utils import run_bass_kernel_spmd
from concourse.library_config import mlp
from concourse.masks import make_identity

P = 128
F32 = mybir.dt.float32
BF16 = mybir.dt.bfloat16
I16 = mybir.dt.int16
BF = ml_dtypes.bfloat16
NSTREAM = 4
GW = 4                    # windows per group (= 512 cols)


class Cfg:
    def __init__(self, n_nodes, n_edges, in_feats, h_feats, n_cores,
                 max_span_tiles=8, mm_chunk=512):
        assert n_nodes % n_cores == 0
        self.n_nodes, self.n_edges = n_nodes, n_edges
        self.in_feats, self.h = in_feats, h_feats
        self.nc = n_cores
        self.shard = n_nodes // n_cores
        self.sp = ((self.shard + P - 1) // P) * P      # padded shard 12544
        self.t = self.sp // P                          # node tiles 98
        self.tp = self.sp + P                          # table rows/core 12672
        # part split: asymmetric window ranges (early parts tiny so the
        # first collective fires early); last part includes the zero tile
        self.npart = 4
        self.pstart = [0, 8, 24, 56]                   # window starts
        self.prow = [(self.pstart[p + 1] - self.pstart[p]) * P
                     for p in range(self.npart - 1)]
        self.prow.append(self.tp - self.pstart[-1] * P)
        self.zero_off = self.sp - self.pstart[-1] * P
        # 2*npart chunks: (part, core-half)
        self.half = n_cores // 2
        self.nchunk = 2 * self.npart
        assert all(self.half * r <= 32640 for r in self.prow)
        self.n_groups = -(-self.t // GW)               # 25
        # part p covers groups [gstart[p], gstart[p+1])
        self.gstart = [w // GW for w in self.pstart] + [self.n_groups]
        self.max_span_tiles = max_span_tiles
        self.mm_chunk = mm_chunk
        assert mm_chunk == GW * P


# ---------------------------------------------------------------- host prep

def _wrap16(x):
    """flat int16 stream -> [128, n/16]: storage[p, col] = x[col*16 + p%16]."""
    assert len(x) % 16 == 0
    return np.tile(x.reshape(-1, 16).T, (8, 1)).copy()


def preprocess(cfg, in_feat, src, dst, W1, b1, W2, b2, W3, b3, W4, b4):
    n = cfg.n_nodes
    deg = np.bincount(dst, minlength=n).astype(np.float32)
    dinv = np.clip(deg, 1.0, None) ** -0.5

    shard_of = dst // cfg.shard
    streams = []
    for c in range(cfg.nc):
        m = shard_of == c
        s_c, d_c = src[m].astype(np.int64), dst[m].astype(np.int64)
        score = s_c // cfg.shard                       # owner core of src
        r = s_c % cfg.shard                            # local node row
        part = np.searchsorted(np.array(cfg.pstart[1:]), r // P,
                               side="right")
        prow = np.array(cfg.prow)[part]
        row = ((score % cfg.half) * prow
               + (r - np.array(cfg.pstart)[part] * P))
        q = part * 2 + score // cfg.half               # chunk index
        dstloc = d_c - c * cfg.shard
        o = np.lexsort((dstloc, q))
        streams.append((row[o], q[o], dstloc[o]))

    # canonical chunk lengths (max over cores, tile-rounded)
    chunk_lens = []
    for cc in range(cfg.nchunk):
        L = max(int((q == cc).sum()) for _, q, _ in streams)
        chunk_lens.append(-(-L // P) * P)
    total_tiles = sum(chunk_lens) // P
    tslots = total_tiles * P

    # per-core slot streams (slot order: chunk-major in chunk index order)
    PADV = 10 ** 9
    pad_row = [cfg.zero_off if cc // 2 == cfg.npart - 1 else 0
               for cc in range(cfg.nchunk)]            # masked anyway
    gz = np.zeros((cfg.nc, tslots), np.int32)
    dl = np.full((cfg.nc, tslots), PADV, np.int64)
    cbase = {}
    base = 0
    for cc in range(cfg.nchunk):
        cbase[cc] = base
        for c in range(cfg.nc):
            row, q, dstloc = streams[c]
            m = q == cc
            k = int(m.sum())
            gz[c, base:base + k] = row[m]
            gz[c, base + k:base + chunk_lens[cc]] = pad_row[cc]
            dl[c, base:base + k] = dstloc[m]
        base += chunk_lens[cc]

    # per-tile window sets (union over cores)
    W = dl // P
    tile_wins = []
    for t in range(total_tiles):
        wins = np.unique(W[:, t * P:(t + 1) * P])
        wins = sorted(int(w) for w in wins if w < cfg.t)
        tile_wins.append(wins)
    nstream = max(len(w) for w in tile_wins)
    assert nstream <= 12, nstream

    # per-chunk bundle lists
    cb = []
    for cc in range(cfg.nchunk):
        t0, t1 = cbase[cc] // P, (cbase[cc] + chunk_lens[cc]) // P
        tiles_of = {}
        for t in range(t0, t1):
            for j, w in enumerate(tile_wins[t]):
                tiles_of.setdefault(w, []).append((t, j))
        bl = []
        b0 = t0
        while b0 < t1:
            bt = min(cfg.max_span_tiles, t1 - b0)
            evs = []
            for w in sorted(tiles_of):
                tl = [(t - b0, j) for (t, j) in tiles_of[w]
                      if b0 <= t < b0 + bt]
                if tl:
                    evs.append((w, tl))
            bl.append((cc, bt, b0, evs))
            b0 += bt
        cb.append(bl)

    # emission order matched to table-part availability: part 0 only,
    # then parts 0-1, then all parts round-robin
    order = []
    ptr = [0] * cfg.nchunk

    def emit(ccs, upto):
        while len(order) < upto and any(ptr[c] < len(cb[c]) for c in ccs):
            for c in ccs:
                if ptr[c] < len(cb[c]):
                    order.append(cb[c][ptr[c]]); ptr[c] += 1

    emit((0, 1), 8)
    emit((0, 1, 2, 3), 16)
    emit(tuple(range(cfg.nchunk)), 10 ** 9)

    # first-touch + last-touch in emission order
    first_seen = set()
    last_touch = {}
    plan = []
    for bi, (cc, bt, b0, evs) in enumerate(order):
        events = []
        for (w, tl) in evs:
            ft = w not in first_seen
            first_seen.add(w)
            last_touch[w] = bi
            events.append((w, tl, ft))
        plan.append([cc, bt, b0, events, []])
    assert len(first_seen) == cfg.t, (len(first_seen), cfg.t)
    # group completion markers
    for g in range(cfg.n_groups):
        wlo, whi = g * GW, min((g + 1) * GW, cfg.t)
        done_at = max(last_touch[w] for w in range(wlo, whi))
        plan[done_at][4].append(g)

    # dstw streams: value of slot relative to its tile's j-th window
    dws = np.full((nstream, cfg.nc, tslots), 999.0, np.float64)
    tl_idx = np.arange(tslots) // P
    for j in range(nstream):
        wj = np.array([tile_wins[t][j] if len(tile_wins[t]) > j else -10
                       for t in range(total_tiles)])
        v = dl - (wj[tl_idx] * P)[None, :]
        dws[j] = np.where((v >= 0) & (v < P) & (dl < PADV), v, 999.0)

    in_maps = []
    for c in range(cfg.nc):
        lo, hi = c * cfg.shard, (c + 1) * cfg.shard
        xT = np.zeros((cfg.in_feats, cfg.sp), BF)
        xT[:, :cfg.shard] = in_feat[lo:hi].T.astype(BF)
        full = np.ones(cfg.sp, np.float32)
        full[:cfg.shard] = dinv[lo:hi]
        dpm = np.ascontiguousarray(full.reshape(cfg.t, P).T)
        # dstw_all [P, nstream, total_tiles]
        da = np.stack([
            np.ascontiguousarray(dws[j, c].reshape(total_tiles, P).T)
            for j in range(nstream)], axis=1).astype(BF)
        in_maps.append({
            "xT": xT, "dinv_pm": dpm,
            "gidx": _wrap16(gz[c].astype(np.int16)),
            "dstw": np.ascontiguousarray(da),
            "W1": np.asarray(W1, BF), "W2": np.asarray(W2, BF),
            "W3": np.asarray(W3, BF), "W4": np.asarray(W4, BF),
            "b1": np.asarray(b1, np.float32).reshape(-1, 1),
            "b2": np.asarray(b2, np.float32).reshape(-1, 1),
            "b3": np.asarray(b3, np.float32).reshape(-1, 1),
            "b4": np.asarray(b4, np.float32).reshape(-1, 1),
        })
    return in_maps, plan, total_tiles, nstream


# ---------------------------------------------------------------- builder

def build_nc(cfg, plan, total_tiles, nstream):
    H = cfg.h
    ST = cfg.max_span_tiles
    idx_cols = total_tiles * 8
    nc = bacc.Bacc("TRN2", target_bir_lowering=False, debug=False,
                   num_devices=cfg.nc)
    xT_d = nc.dram_tensor("xT", [cfg.in_feats, cfg.sp], BF16, kind="ExternalInput")
    dinv_d = nc.dram_tensor("dinv_pm", [P, cfg.t], F32, kind="ExternalInput")
    gidx_d = nc.dram_tensor("gidx", [P, idx_cols], I16, kind="ExternalInput")
    dstw_d = nc.dram_tensor("dstw", [P, nstream, total_tiles], BF16,
                            kind="ExternalInput")
    W_d = {w: nc.dram_tensor(w, [cfg.in_feats if w in ("W1", "W4") else H, H],
                             BF16, kind="ExternalInput")
           for w in ("W1", "W2", "W3", "W4")}
    b_d = {b: nc.dram_tensor(b, [H, 1], F32, kind="ExternalInput")
           for b in ("b1", "b2", "b3", "b4")}
    outl_d = nc.dram_tensor("out_l", [H, cfg.sp], F32, kind="ExternalOutput")
    outh_d = nc.dram_tensor("out_h", [H, cfg.sp], F32, kind="ExternalOutput")

    relu = mybir.ActivationFunctionType.Relu
    cp = mybir.ActivationFunctionType.Copy

    with tile.TileContext(nc) as tc, ExitStack() as ctx:
        pers = ctx.enter_context(tc.tile_pool(name="pers", bufs=1))
        dram = ctx.enter_context(tc.tile_pool(name="dram", bufs=1, space="DRAM"))
        io = ctx.enter_context(tc.tile_pool(name="io", bufs=2))
        idxp = ctx.enter_context(tc.tile_pool(name="idxp", bufs=8))
        gbp = ctx.enter_context(tc.tile_pool(name="gbp", bufs=8))
        gbi = ctx.enter_context(tc.tile_pool(name="gbi", bufs=4))
        psum = ctx.enter_context(tc.tile_pool(name="psum", bufs=2, space="PSUM"))
        psum1 = ctx.enter_context(tc.tile_pool(name="psum1", bufs=2, space="PSUM"))
        psum2 = ctx.enter_context(tc.tile_pool(name="psum2", bufs=2, space="PSUM"))

        nc.gpsimd.load_library(mlp)

        f0 = pers.tile([P, cfg.t, 64], BF16, tag="f0")
        f1 = pers.tile([P, cfg.t, 64], BF16, tag="f1")
        f2 = pers.tile([P, cfg.t, 64], BF16, tag="f2")
        tbl = pers.tile([P, cfg.t + 1, 128], BF16, tag="tbl")
        dinv_s = pers.tile([P, cfg.t], F32, tag="dinv")
        dinv_bf = pers.tile([P, cfg.t], BF16, tag="dinv_bf")
        Ws = {w: pers.tile([cfg.in_feats if w in ("W1", "W4") else H, H],
                           BF16, tag=w, name=w + "_s")
              for w in ("W1", "W2", "W3", "W4")}
        bs = {b: pers.tile([H, 1], F32, tag=b, name=b + "_s")
              for b in ("b1", "b2", "b3", "b4")}
        ident = pers.tile([P, P], BF16, tag="ident")
        sid3 = pers.tile([P, P], BF16, tag="sid3")
        sid075 = pers.tile([P, P], BF16, tag="sid075")
        sidm15 = pers.tile([P, P], BF16, tag="sidm15")
        agg = pers.tile([P, cfg.t, 64], F32, tag="agg")
        iota_f = pers.tile([P, P], BF16, tag="iota_f")

        # DRAM tables: per round, npart parts
        tb_in = [[dram.tile([cfg.prow[p], 128], BF16, name=f"tb_in{r}p{p}")
                  for p in range(cfg.npart)] for r in range(2)]
        tb_full = [[dram.tile([cfg.prow[p] * cfg.nc, 128], BF16,
                              addr_space="Shared", name=f"tb_full{r}p{p}")
                    for p in range(cfg.npart)] for r in range(2)]

        for w in Ws:
            nc.sync.dma_start(Ws[w][:], W_d[w][:])
        for b in bs:
            nc.sync.dma_start(bs[b][:], b_d[b][:])
        nc.sync.dma_start(dinv_s[:], dinv_d[:])
        make_identity(nc, ident[:])
        nc.vector.tensor_scalar_mul(sid3[:], ident[:], 3.0)
        nc.vector.tensor_scalar_mul(sid075[:], ident[:], 0.75)
        nc.vector.tensor_scalar_mul(sidm15[:], ident[:], -1.5)
        nc.gpsimd.memset(tbl[:, cfg.t, :], 0.0)
        ioti = pers.tile([P, P], mybir.dt.int32, tag="ioti")
        nc.gpsimd.iota(ioti[:], pattern=[[1, P]], base=0, channel_multiplier=0)
        nc.vector.tensor_copy(iota_f[:], ioti[:])
        nc.vector.tensor_copy(dinv_bf[:], dinv_s[:])
        # force the mlp library reload off the critical path: a throwaway
        # 128-idx gather (reads garbage, output unused) makes the framework
        # reload IRAM now, and a 1-row AllGather absorbs inter-core launch
        # skew before the first real collective.
        gi0 = pers.tile([P, 8], I16, tag="gi0")
        nc.vector.memset(gi0[:], 0)
        warm = pers.tile([P, 1, 128], BF16, tag="warm")
        nc.gpsimd.dma_gather(warm[:], tb_full[0][0][0:P, :], gi0[:],
                             P, P, 128)
        # zero rows of both rounds' last-part tables
        for r in range(2):
            nc.sync.dma_start(
                tb_in[r][-1][cfg.zero_off:cfg.zero_off + P, :]
                .rearrange("(t p) f -> p t f", p=P),
                tbl[:, cfg.t:cfg.t + 1, :])

        def table_group(rnd, fsrc, g):
            """tbl[:, grp] = dinv*f; DMA to tb_in[rnd]; collectives at marks."""
            wlo = g * GW
            whi = min(wlo + GW, cfg.t)
            nw = whi - wlo
            nc.vector.tensor_tensor(
                tbl[:, wlo:whi, :64], fsrc[:, wlo:whi, :],
                dinv_bf[:, wlo:whi, None].to_broadcast([P, nw, 64]),
                mybir.AluOpType.mult)
            p = 0
            while p + 1 < cfg.npart and cfg.pstart[p + 1] <= whi - 1:
                p += 1
            off = wlo * P - cfg.pstart[p] * P
            nc.sync.dma_start(
                tb_in[rnd][p][off:off + nw * P, :]
                .rearrange("(t p) f -> p t f", p=P),
                tbl[:, wlo:whi, :])

        def collective(rnd, p):
            nc.gpsimd.collective_compute(
                "AllGather", mybir.AluOpType.bypass,
                replica_groups=[list(range(cfg.nc))],
                ins=[tb_in[rnd][p][:]], outs=[tb_full[rnd][p][:]])

        # chunk -> (tb_full part, base row within part table)
        def chunk_region(rnd, cc):
            p = cc // 2
            half = cc % 2
            rows = cfg.prow[p]
            base = half * cfg.half * rows
            return tb_full[rnd][p], base, cfg.half * rows

        # per part: groups of that part; fire bundle index (all groups done)
        part_groups = [list(range(cfg.gstart[p], cfg.gstart[p + 1]))
                       for p in range(cfg.npart)]
        fire_at = {}
        seen_g = set()
        for bi, (_, _, _, _, gdone) in enumerate(plan):
            seen_g.update(gdone)
            for p in range(cfg.npart - 1):
                if p not in fire_at and all(g in seen_g
                                            for g in part_groups[p]):
                    fire_at[p] = bi + 2          # +2: input DMAs drained
        assert len(fire_at) == cfg.npart - 1
        first_need = {}
        for bi, (cc, _, _, _, _) in enumerate(plan):
            first_need.setdefault(cc // 2, bi)

        # ---- phase 1: MLP -> f0 node-major; table 0 produced per group
        CH = cfg.mm_chunk
        n_mlp = -(-cfg.sp // CH)
        for k in range(n_mlp):
            j0 = k * CH
            w = min(CH, cfg.sp - j0)
            xc = io.tile([cfg.in_feats, CH], BF16, tag="xc")
            nc.sync.dma_start(xc[:, :w], xT_d[:, j0:j0 + w])
            ps1 = psum.tile([H, CH], F32, tag="A")
            nc.tensor.matmul(ps1[:, :w], Ws["W1"][:], xc[:, :w],
                             start=True, stop=True)
            h1c = io.tile([H, CH], BF16, tag="h1c")
            nc.scalar.activation(h1c[:, :w], ps1[:, :w], relu, bias=bs["b1"][:])
            ps2 = psum.tile([H, CH], F32, tag="B")
            nc.tensor.matmul(ps2[:, :w], Ws["W2"][:], h1c[:, :w],
                             start=True, stop=True)
            h2c = io.tile([H, CH], BF16, tag="h2c")
            nc.scalar.activation(h2c[:, :w], ps2[:, :w], relu, bias=bs["b2"][:])
            for i in range(w // P):
                t = (j0 + i * P) // P
                ps3 = psum1.tile([P, 64], BF16, tag="C")
                nc.tensor.transpose(ps3[:], h2c[:, i * P:(i + 1) * P],
                                    ident[:H, :H])
                nc.scalar.activation(f0[:, t, :], ps3[:], cp)
            table_group(0, f0, k)
            for p in range(cfg.npart - 1):
                if k == cfg.gstart[p + 1] - 1 and p < 2:
                    collective(0, p)

        # ---- message passing rounds
        for rnd, (fprev, fnext) in enumerate([(f0, f1), (f1, f2)]):
            for bi, (cc, btiles, goff, events, gdone) in enumerate(plan):
                if rnd == 0:
                    for p in (2, 3):
                        if first_need[p] == bi:
                            collective(0, p)
                tbf, cbase_, clen = chunk_region(rnd, cc)
                gi = idxp.tile([P, ST * 8], I16, tag="gi")
                nc.sync.dma_start(gi[:, :btiles * 8],
                                  gidx_d[:, goff * 8:(goff + btiles) * 8])
                dv = idxp.tile([P, nstream, ST], BF16, tag="dv")
                nc.scalar.dma_start(dv[:, :, :btiles],
                                    dstw_d[:, :, goff:goff + btiles])
                gb = gbp.tile([P, ST, 128], BF16, tag="gb")
                ni = btiles * P
                # byte-mover bitcast: uint8 descriptors measure fastest on
                # the SWDGE gather ucode (same 256B/row moved)
                nc.gpsimd.dma_gather(
                    gb[:, :btiles, :].bitcast(mybir.dt.uint8),
                    tbf[cbase_:cbase_ + clen, :].bitcast(mybir.dt.uint8),
                    gi[:, :btiles * 8], ni, ni, 256)
                for (ww, tl, ft) in events:
                    ind = gbi.tile([P, ST, P], BF16, tag="ind")
                    i0 = 0
                    while i0 < len(tl):
                        t0, j0_ = tl[i0]
                        i1 = i0 + 1
                        while (i1 < len(tl) and tl[i1][1] == j0_
                               and tl[i1][0] == tl[i1 - 1][0] + 1):
                            i1 += 1
                        ln = i1 - i0
                        nc.vector.tensor_tensor(
                            ind[:, i0:i1, :],
                            iota_f[:, None, :].to_broadcast([P, ln, P]),
                            dv[:, j0_, t0:t0 + ln, None].to_broadcast(
                                [P, ln, P]),
                            mybir.AluOpType.is_equal)
                        i0 = i1
                    pw = psum2.tile([P, 64], F32, tag="D")
                    for i, (t, j) in enumerate(tl):
                        nc.tensor.matmul(pw[:], ind[:, i, :],
                                         gb[:, t, :64],
                                         start=(i == 0), stop=(i == len(tl) - 1))
                    if ft:
                        nc.vector.tensor_copy(agg[:, ww, :], pw[:])
                    else:
                        nc.vector.tensor_tensor(agg[:, ww, :], agg[:, ww, :],
                                                pw[:], mybir.AluOpType.add)
                for g in gdone:
                    wlo = g * GW
                    whi = min(wlo + GW, cfg.t)
                    nw = whi - wlo
                    # fnext = fprev - dinv*agg
                    nc.vector.tensor_tensor(
                        fnext[:, wlo:whi, :], agg[:, wlo:whi, :],
                        dinv_s[:, wlo:whi, None].to_broadcast([P, nw, 64]),
                        mybir.AluOpType.mult)
                    nc.vector.tensor_tensor(
                        fnext[:, wlo:whi, :], fprev[:, wlo:whi, :],
                        fnext[:, wlo:whi, :], mybir.AluOpType.subtract)
                    if rnd == 0:
                        # f0 := f0 - f1 (filters); round-2 table from f1
                        nc.vector.tensor_tensor(
                            f0[:, wlo:whi, :], f0[:, wlo:whi, :],
                            f1[:, wlo:whi, :], mybir.AluOpType.subtract)
                        table_group(1, f1, g)
                    else:
                        # filters + output MLP, in 2-window slices
                        for s0 in range(wlo, whi, 2):
                            s1 = min(s0 + 2, whi)
                            ns_ = s1 - s0
                            j0 = s0 * P
                            w = ns_ * P
                            CH2 = 2 * P
                            zl = psum.tile([H, CH2], F32, tag="A")
                            z1 = psum.tile([H, CH2], F32, tag="B")
                            z2 = psum1.tile([H, CH2], F32, tag="C")
                            for i in range(ns_):
                                t = s0 + i
                                cs = slice(i * P, (i + 1) * P)
                                nc.tensor.matmul(zl[:, cs], f0[:, t, :],
                                                 sid3[:], start=True,
                                                 stop=False)
                                nc.tensor.matmul(zl[:, cs], f2[:, t, :],
                                                 sid075[:], start=False,
                                                 stop=True)
                                nc.tensor.matmul(z1[:, cs], f1[:, t, :],
                                                 sid3[:], start=True,
                                                 stop=False)
                                nc.tensor.matmul(z1[:, cs], f2[:, t, :],
                                                 sidm15[:], start=False,
                                                 stop=True)
                                nc.tensor.matmul(z2[:, cs], f2[:, t, :],
                                                 sid075[:], start=True,
                                                 stop=True)
                            zlc = io.tile([H, CH2], BF16, tag="zlc")
                            zhc = io.tile([P, CH2], BF16, tag="zhc")
                            nc.scalar.activation(zlc[:, :w], zl[:, :w], cp)
                            nc.scalar.activation(zhc[:H, :w], z1[:, :w], cp)
                            nc.scalar.activation(zhc[H:, :w], z2[:, :w], cp)
                            pl = psum1.tile([H, CH2], F32, tag="C")
                            ph = psum.tile([H, CH2], F32, tag="A")
                            nc.tensor.matmul(pl[:, :w], Ws["W3"][:],
                                             zlc[:, :w], start=True,
                                             stop=True)
                            nc.tensor.matmul(ph[:, :w], Ws["W4"][:],
                                             zhc[:, :w], start=True,
                                             stop=True)
                            ol = io.tile([H, CH2], F32, tag="ol")
                            oh = io.tile([H, CH2], F32, tag="oh")
                            nc.scalar.activation(ol[:, :w], pl[:, :w], relu,
                                                 bias=bs["b3"][:])
                            nc.scalar.activation(oh[:, :w], ph[:, :w], relu,
                                                 bias=bs["b4"][:])
                            nc.sync.dma_start(outl_d[:, j0:j0 + w],
                                              ol[:, :w])
                            nc.sync.dma_start(outh_d[:, j0:j0 + w],
                                              oh[:, :w])
                if rnd == 0:
                    for p in range(cfg.npart - 1):
                        if fire_at[p] == bi:
                            collective(1, p)

            if rnd == 0:
                collective(1, cfg.npart - 1)

    nc.compile()
    return nc


# ---------------------------------------------------------------- driver

_CACHE = {}


def run(cfg, inputs, run_fn=None, **spmd_kwargs):
    in_maps, plan, total_tiles, nstream = preprocess(cfg, **inputs)
    key = (cfg.n_nodes, cfg.n_edges, cfg.nc, cfg.max_span_tiles,
           total_tiles, nstream, repr(plan))
    if key not in _CACHE:
        _CACHE[key] = build_nc(cfg, plan, total_tiles, nstream)
    nc = _CACHE[key]
    if run_fn is not None:
        results = run_fn(nc, in_maps)
        res = None
    else:
        res = run_bass_kernel_spmd(nc, in_maps, core_ids=list(range(cfg.nc)),
                                   **spmd_kwargs)
        results = res.results
    h_l = np.zeros((cfg.n_nodes, cfg.h), np.float32)
    h_h = np.zeros((cfg.n_nodes, cfg.h), np.float32)
    for c in range(cfg.nc):
        lo = c * cfg.shard
        h_l[lo:lo + cfg.shard] = results[c]["out_l"].T[:cfg.shard]
        h_h[lo:lo + cfg.shard] = results[c]["out_h"].T[:cfg.shard]
    return h_l, h_h, res


def kernel(in_feat, src, dst, W1, b1, W2, b2, W3, b3, W4, b4):
    cfg = Cfg(100000, 1600000, 128, 64, 8)
    h_l, h_h, _ = run(cfg, dict(
        in_feat=np.asarray(in_feat, np.float32),
        src=np.asarray(src, np.int64), dst=np.asarray(dst, np.int64),
        W1=np.asarray(W1, np.float32), b1=np.asarray(b1, np.float32),
        W2=np.asarray(W2, np.float32), b2=np.asarray(b2, np.float32),
        W3=np.asarray(W3, np.float32), b3=np.asarray(b3, np.float32),
        W4=np.asarray(W4, np.float32), b4=np.asarray(b4, np.float32)))
    return h_l, h_h
